# revision 39
# baseline (speedup 1.0000x reference)
"""DiT block kernel for Trainium2 (8 NeuronCores, data-parallel over batch).

Reference computation (per batch b):
    c = silu(cond) @ w_ada + b_ada
    shift_msa, scale_msa, gate_msa, shift_mlp, scale_mlp, gate_mlp = split(c)
    h  = LN1(x) * (1+scale_msa) + shift_msa
    x  = x + gate_msa * (attn(h) @ wo + bo)
    h2 = LN2(x) * (1+scale_mlp) + shift_mlp
    x  = x + gate_mlp * (silu(h2 @ w1 + b1) @ w2 + b2)

Per-core layout (8 batches, deeply software-pipelined):
  - matmul activations feature-major bf16 [128 = feat%128, 6, L]
  - attention per (head, q-half): S^T = K Q^T, two ko-tiles packed into one
    [128,1024] PSUM pair, one wide exp on ACT, AV with an appended ones
    column accumulating the softmax denominator -> per-(h,nh) reciprocal
    + DMA partition-broadcast + normalize multiply.
  - V bias folded into the out-projection bias (rows of attn sum to 1).
  - all non-attention work (LN prologue of b+1, out-proj/LN2/MLP of b-1)
    is emitted as ~45 "filler" units interleaved into attention(b) so the
    PE never sees a phase boundary; Q/K/V runs as a PE-dense block between
    attention phases.
"""

import numpy as np

HID = 768
HEADS = 12
HD = 64
B, L_FULL = 64, 1024
NCORES = 8
NB = B // NCORES
EPS = 1e-6

_PROJ_CHUNKS = ((0, 512), (512, 256))


def build_nc(nb=NB, L=L_FULL):
    import concourse.mybir as mybir
    import concourse.tile as tile
    from concourse import bacc

    f32 = mybir.dt.float32
    bf16 = mybir.dt.bfloat16

    nc = bacc.Bacc("TRN2", target_bir_lowering=False, debug=False)

    io = {}
    io["x_img"] = nc.dram_tensor("x_img", [nb, L, HID], f32, kind="ExternalInput")
    io["cond"] = nc.dram_tensor("cond", [nb, HID], f32, kind="ExternalInput")
    for name in ("wq", "wk", "wv", "wo", "w1", "w2"):
        io[name] = nc.dram_tensor(name, [HID, HID], f32, kind="ExternalInput")
    for name in ("bq", "bk", "bv", "bo", "b1", "b2"):
        io[name] = nc.dram_tensor(name, [HID], f32, kind="ExternalInput")
    io["w_ada"] = nc.dram_tensor("w_ada", [HID, 6 * HID], f32, kind="ExternalInput")
    io["b_ada"] = nc.dram_tensor("b_ada", [6 * HID], f32, kind="ExternalInput")
    for name in ("ln1_scale", "ln1_bias", "ln2_scale", "ln2_bias"):
        io[name] = nc.dram_tensor(name, [HID], f32, kind="ExternalInput")
    io["out"] = nc.dram_tensor("out", [nb, L, HID], f32, kind="ExternalOutput")
    io["c_dram"] = nc.dram_tensor("c_scratch", [nb, 6 * HID], f32)
    io["x2_dram"] = nc.dram_tensor("x2_scratch", [nb, L, HID], bf16)
    io["den_dram"] = nc.dram_tensor("den_scratch", [nb, HEADS, L], bf16)
    io["rec_dram"] = nc.dram_tensor("rec_scratch", [nb, HEADS, L], bf16)

    with tile.TileContext(nc) as tc:
        _build(tc, nc, io, nb, L)
    nc.compile()
    return nc


def _build(tc, nc, io, nb, L):
    import contextlib
    from collections import deque

    import concourse.mybir as mybir
    from concourse.masks import make_identity

    dt = mybir.dt
    f32, bf16 = dt.float32, dt.bfloat16
    AF = mybir.ActivationFunctionType
    OP = mybir.AluOpType

    LO = L // 128
    KO = HID // 128
    NADA = 6 * HID

    ctx = contextlib.ExitStack()
    with ctx:
        consts = ctx.enter_context(tc.tile_pool(name="consts", bufs=1))
        wpool = ctx.enter_context(tc.tile_pool(name="wpool", bufs=1))
        rows = ctx.enter_context(tc.tile_pool(name="rows", bufs=1))
        xs = ctx.enter_context(tc.tile_pool(name="xs", bufs=2))
        xhp = ctx.enter_context(tc.tile_pool(name="xhp", bufs=1))
        fmA = ctx.enter_context(tc.tile_pool(name="fmA", bufs=1))
        fmB = ctx.enter_context(tc.tile_pool(name="fmB", bufs=2))
        fmC = ctx.enter_context(tc.tile_pool(name="fmC", bufs=2))
        qkv = ctx.enter_context(tc.tile_pool(name="qkv", bufs=1))
        ptp = ctx.enter_context(tc.tile_pool(name="ptp", bufs=2))
        rbp = ctx.enter_context(tc.tile_pool(name="rbp", bufs=2))
        gmp = ctx.enter_context(tc.tile_pool(name="gmp", bufs=2))
        outp = ctx.enter_context(tc.tile_pool(name="outp", bufs=1))
        small = ctx.enter_context(tc.tile_pool(name="small", bufs=2))
        ps = ctx.enter_context(tc.tile_pool(name="ps", bufs=4, space="PSUM"))

        _psc = [0]

        def psum_b(w=512):
            _psc[0] += 1
            t = ps.tile([128, 512], f32, tag="ps_b", bufs=2, name=f"psb{_psc[0]}")
            return t[:, :w] if w != 512 else t

        def psum_s():
            _psc[0] += 1
            return ps.tile([128, 1024], f32, tag="ps_s", bufs=2, name=f"pss{_psc[0]}")

        def psum_av():
            _psc[0] += 1
            return ps.tile([128, 512], f32, tag="ps_av", bufs=2, name=f"psav{_psc[0]}")

        def stage2k():
            # 2KB/partition staging tiles time-share the attention pt slots
            _psc[0] += 1
            return ptp.tile([128, 512], f32, tag="PT", name=f"stg{_psc[0]}")

        # ---- constants ----
        id_bf = consts.tile([128, 128], bf16)
        make_identity(nc, id_bf)
        id_f32 = consts.tile([128, 128], f32)
        make_identity(nc, id_f32)
        ones_bf = consts.tile([1, 128], bf16)
        nc.vector.memset(ones_bf, 1.0)
        ones_f32 = consts.tile([1, 128], f32)
        nc.vector.memset(ones_f32, 1.0)
        ones_col_bf = consts.tile([128, 1], bf16)
        nc.vector.memset(ones_col_bf, 1.0)
        # constant subtracted inside exp (cancels in softmax)
        negc_col = consts.tile([128, 1], f32)
        nc.vector.memset(negc_col, -10.0)

        def load_fm(dram_vec):
            t = consts.tile([128, KO], f32, tag=f"fm_{dram_vec.name}")
            with nc.allow_non_contiguous_dma(reason="small 1d fm load"):
                nc.sync.dma_start(out=t, in_=dram_vec.ap().rearrange("(ko p) -> p ko", p=128))
            return t

        ln1s_fm = load_fm(io["ln1_scale"])
        ln1b_fm = load_fm(io["ln1_bias"])
        ln2s_fm = load_fm(io["ln2_scale"])
        ln2b_fm = load_fm(io["ln2_bias"])
        bq_fm = load_fm(io["bq"])
        bk_fm = load_fm(io["bk"])
        bv_fm = load_fm(io["bv"])
        b1_fm = load_fm(io["b1"])
        b1h_fm = consts.tile([128, KO], f32)  # 0.5 * b1, bias for tanh(x/2)
        nc.vector.tensor_scalar_mul(out=b1h_fm, in0=b1_fm, scalar1=0.5)
        bv_bfc = consts.tile([128, KO], bf16)  # bf16 feature-major bv column
        nc.vector.tensor_copy(out=bv_bfc, in_=bv_fm)

        bo_r32 = rows.tile([1, HID], f32, tag="row_f32", name="bo_r32")
        nc.sync.dma_start(out=bo_r32, in_=io["bo"].ap()[None, :])
        bo_rowb = consts.tile([1, HID], bf16, tag="bo_rowb")
        nc.vector.tensor_copy(out=bo_rowb, in_=bo_r32)
        b2_r32 = rows.tile([1, HID], f32, tag="row_f32", name="b2_r32")
        nc.sync.dma_start(out=b2_r32, in_=io["b2"].ap()[None, :])
        b2_row = consts.tile([1, HID], bf16, tag="b2_row")
        nc.vector.tensor_copy(out=b2_row, in_=b2_r32)

        # ---- conditioning: scT = silu(cond)^T [128, KO, nb] via a
        # transposed (feature-major) DMA load of the small cond tensor ----
        condT = consts.tile([128, KO, nb], f32)
        condT_view = io["cond"].ap().rearrange("b (ko p) -> p ko b", p=128)
        with nc.allow_non_contiguous_dma(reason="small cond transposed load"):
            for kf in range(KO):
                nc.sync.dma_start(out=condT[:, kf, :], in_=condT_view[:, kf, :])
        sgT = consts.tile([128, KO, nb], f32)
        nc.scalar.activation(out=sgT, in_=condT, func=AF.Sigmoid)
        scT = consts.tile([128, KO, nb], f32)
        nc.vector.tensor_mul(out=scT, in0=condT, in1=sgT)

        # ---- c = silu(cond) @ w_ada + b_ada  -> c_dram + feature-major cT ----
        cT = consts.tile([128, 6 * KO, nb], f32)
        wada_view = io["w_ada"].ap().rearrange("(ko p) n -> p ko n", p=128)
        for jc in range(NADA // 512):
            bst = rows.tile([1, 512], f32, tag="row_f32")
            nc.sync.dma_start(out=bst, in_=io["b_ada"].ap()[None, jc * 512:(jc + 1) * 512])
            pc = psum_b()
            for kf in range(KO):
                wst = stage2k()
                nc.sync.dma_start(out=wst,
                                  in_=wada_view[:, kf, jc * 512:(jc + 1) * 512])
                nc.tensor.matmul(pc[:nb, :], lhsT=scT[:, kf, :], rhs=wst,
                                 start=(kf == 0), stop=False)
            nc.tensor.matmul(pc[:nb, :], lhsT=ones_f32[:, :nb],
                             rhs=bst, start=False, stop=True)
            cst = stage2k()[:nb, :]
            nc.vector.tensor_copy(out=cst, in_=pc[:nb, :])
            nc.sync.dma_start(out=io["c_dram"].ap()[:, jc * 512:(jc + 1) * 512], in_=cst)
            for mt in range(4):
                mo = jc * 4 + mt
                ptr = psum_b()
                nc.tensor.transpose(ptr[:, :nb], cst[:, mt * 128:(mt + 1) * 128],
                                    id_f32[:nb, :nb])
                nc.vector.tensor_copy(out=cT[:, mo, :], in_=ptr[:, :nb])

        def chunk(i):
            return cT[:, 6 * i:6 * i + 6, :]

        a1 = consts.tile([128, KO, nb], f32)
        c1 = consts.tile([128, KO, nb], f32)
        a2 = consts.tile([128, KO, nb], f32)
        c2 = consts.tile([128, KO, nb], f32)
        tmp_m = consts.tile([128, KO, nb], f32)
        for (a, c, lns, lnb, sc_i, sh_i) in ((a1, c1, ln1s_fm, ln1b_fm, 1, 0),
                                             (a2, c2, ln2s_fm, ln2b_fm, 4, 3)):
            nc.vector.tensor_scalar_add(out=tmp_m, in0=chunk(sc_i), scalar1=1.0)
            nc.vector.tensor_mul(out=a, in0=tmp_m,
                                 in1=lns[:, :, None].to_broadcast([128, KO, nb]))
            nc.vector.tensor_mul(out=c, in0=tmp_m,
                                 in1=lnb[:, :, None].to_broadcast([128, KO, nb]))
            nc.vector.tensor_add(out=c, in0=c, in1=chunk(sh_i))

        # ---- weights -> SBUF bf16 [128, KO, 768]; q/k/v first ----
        w_bf = {}
        for name in ("wq", "wk", "wv", "wo", "w1", "w2"):
            wt = wpool.tile([128, KO, HID], bf16, tag=f"w_{name}")
            w_view = io[name].ap().rearrange("(ko p) n -> p ko n", p=128)
            for kf in range(KO):
                for (c0, cw) in _PROJ_CHUNKS:
                    st = stage2k()
                    nc.sync.dma_start(out=st[:, :cw], in_=w_view[:, kf, c0:c0 + cw])
                    nc.vector.tensor_copy(out=wt[:, kf, c0:c0 + cw], in_=st[:, :cw])
            w_bf[name] = wt

        # ---- bo' = bv @ wo + bo (attention rows sum to 1, so the V bias
        # passes through AV untouched and folds into the out-proj bias) ----
        bo2_row = consts.tile([1, HID], bf16, tag="bo2_row")
        for (c0, cw) in _PROJ_CHUNKS:
            p = psum_b(cw)
            for kf in range(KO):
                nc.tensor.matmul(p[:1, :], lhsT=bv_bfc[:, kf:kf + 1],
                                 rhs=w_bf["wo"][:, kf, c0:c0 + cw],
                                 start=(kf == 0), stop=False)
            nc.tensor.matmul(p[:1, :], lhsT=ones_bf[:, :1],
                             rhs=bo_rowb[:, c0:c0 + cw], start=False, stop=True)
            nc.vector.tensor_copy(out=bo2_row[:, c0:c0 + cw], in_=p[:1, :])

        # ---- helpers ----
        def rsqrt_newton(dst, var_ap, n):
            """dst[:, :n] = 1/sqrt(var + EPS); Newton from seed 1.0 (var~1)."""
            vt = small.tile([128, 8], f32, tag="rs_v")
            nc.vector.tensor_scalar_add(out=vt[:, :n], in0=var_ap, scalar1=EPS)
            hv = small.tile([128, 8], f32, tag="rs_h")
            nc.vector.tensor_scalar_mul(out=hv[:, :n], in0=vt[:, :n], scalar1=0.5)
            nc.vector.memset(dst[:, :n], 1.0)
            tt = small.tile([128, 8], f32, tag="rs_t")
            for _ in range(5):
                nc.vector.tensor_mul(out=tt[:, :n], in0=dst[:, :n], in1=dst[:, :n])
                nc.vector.tensor_mul(out=tt[:, :n], in0=tt[:, :n], in1=hv[:, :n])
                nc.vector.tensor_scalar(out=tt[:, :n], in0=tt[:, :n],
                                        scalar1=-1.0, scalar2=1.5, op0=OP.mult, op1=OP.add)
                nc.vector.tensor_mul(out=dst[:, :n], in0=dst[:, :n], in1=tt[:, :n])

        def ln_stats(src, mv, lo):
            stats = small.tile([128, 3, 6], f32, tag="stats")
            for s in range(3):
                nc.vector.bn_stats(out=stats[:, s, :], in_=src[:, s * 256:(s + 1) * 256])
            nc.vector.bn_aggr(out=mv[:, lo, :], in_=stats)

        x_view = io["x_img"].ap().rearrange("b (lo p) d -> b p lo d", p=128)
        out_view = io["out"].ap().rearrange("b (lo p) d -> b p lo d", p=128)
        x2_view = io["x2_dram"].ap().rearrange("b (lo p) d -> b p lo d", p=128)

        # ================= per-batch unit generators =================

        def make_pro_units(b):
            """LN1 prologue for batch b. Every DMA is issued one unit ahead
            of the compute that reads it, so DVE ops never wait at the queue
            head (which would block later-queued PSUM drains and stall PE)."""
            st = {"xt": {}}

            def gates_u():
                g_bc = {}
                for gi, nm in ((2, "gmsa"), (5, "gmlp")):
                    gr = rows.tile([1, HID], f32, tag="row_f32", name=f"gr_{b}_{nm}")
                    nc.sync.dma_start(out=gr, in_=io["c_dram"].ap()[b:b + 1, gi * HID:(gi + 1) * HID])
                    grb = rows.tile([1, HID], bf16, tag="growb", name=f"grb_{b}_{nm}")
                    nc.vector.tensor_copy(out=grb, in_=gr)
                    gb = small.tile([128, HID], bf16, tag=f"gbc_{nm}", bufs=2,
                                    name=f"gb_{b}_{nm}")
                    nc.gpsimd.partition_broadcast(gb, grb, channels=128)
                    g_bc[nm] = gb
                st["g"] = g_bc

            def dma_x(lo, pfx):
                t = xs.tile([128, HID], f32, tag="xsl", bufs=3, name=f"{pfx}_{b}_{lo}")
                nc.sync.dma_start(out=t, in_=x_view[b, :, lo, :])
                st["xt"][lo] = t

            def pfx_u():
                st["mv1"] = small.tile([128, LO, 2], f32, tag="mv1", name=f"mv1_{b}")
                st["rstd1"] = small.tile([128, 8], f32, tag="rstd1", name=f"rstd1_{b}")
                dma_x(0, "xs")
                dma_x(1, "xs")

            def st_u(k):
                for lo in (2 * k, 2 * k + 1):
                    ln_stats(st["xt"].pop(lo), st["mv1"], lo)
                if k < 3:
                    dma_x(2 * k + 2, "xs")
                    dma_x(2 * k + 3, "xs")
                if k == 1:
                    rsqrt_newton(st["rstd1"][:, 0:], st["mv1"][:, 0:4, 1], 4)
                if k == 3:
                    rsqrt_newton(st["rstd1"][:, 4:], st["mv1"][:, 4:8, 1], 4)

            def xh_u(k):
                # k=0: prefetch only; k>=1: xhat(2k-2, 2k-1) + prefetch next
                if k >= 1:
                    g = (k - 1) // 2
                    if (k - 1) % 2 == 0:
                        st["xhg"] = xhp.tile([128, 4, HID], bf16, tag="xhg",
                                             name=f"xhg_{b}_{g}")
                    xg = st["xhg"]
                    for i in range(2):
                        lo = 2 * (k - 1) + i
                        nc.vector.tensor_scalar(out=xg[:, lo - 4 * g, :],
                                                in0=st["xt"].pop(lo),
                                                scalar1=st["mv1"][:, lo, 0:1],
                                                scalar2=st["rstd1"][:, lo:lo + 1],
                                                op0=OP.subtract, op1=OP.mult)
                if k < 4:
                    dma_x(2 * k, "xh")
                    dma_x(2 * k + 1, "xh")

            def hT_u(g, half):
                if "hT" not in st:
                    st["hT"] = fmA.tile([128, KO, L], bf16, tag="fmA", name=f"hT_{b}")
                xg, hT = st["xhg"], st["hT"]
                for kf in range(3 * half, 3 * half + 3):
                    p = psum_b().bitcast(bf16)
                    for i in range(4):
                        nc.tensor.transpose(p[:, i * 128:(i + 1) * 128],
                                            xg[:, i, kf * 128:(kf + 1) * 128], id_bf)
                    nc.vector.tensor_scalar(
                        out=hT[:, kf, g * 512:(g + 1) * 512], in0=p[:, :512],
                        scalar1=a1[:, kf, b:b + 1], scalar2=c1[:, kf, b:b + 1],
                        op0=OP.mult, op1=OP.add)

            units = [(0.97, gates_u), (0.02, pfx_u),
                     (0.05, lambda: st_u(0)), (0.08, lambda: st_u(1)),
                     (0.11, lambda: st_u(2)), (0.14, lambda: st_u(3)),
                     (0.16, lambda: xh_u(0)), (0.18, lambda: xh_u(1)),
                     (0.21, lambda: xh_u(2)),
                     (0.24, lambda: hT_u(0, 0)), (0.27, lambda: hT_u(0, 1)),
                     (0.30, lambda: xh_u(3)), (0.33, lambda: xh_u(4)),
                     (0.36, lambda: hT_u(1, 0)), (0.39, lambda: hT_u(1, 1))]
            return st, units

        def emit_qkv(b, pro_st):
            """Q/K/V projections for batch b (PE-dense block)."""
            hT = pro_st["hT"]
            QT = qkv.tile([128, KO, L], bf16, tag="QT", name=f"QT_{b}")
            KT = qkv.tile([128, KO, L], bf16, tag="KT", name=f"KT_{b}")
            for (dst, wname, bfm) in ((QT, "wq", bq_fm), (KT, "wk", bk_fm)):
                wt = w_bf[wname]
                for mo in range(KO):
                    p = psum_s()
                    for nh in range(2):
                        for kf in range(KO):
                            nc.tensor.matmul(p[:, nh * 512:(nh + 1) * 512],
                                             lhsT=wt[:, kf, mo * 128:(mo + 1) * 128],
                                             rhs=hT[:, kf, nh * 512:(nh + 1) * 512],
                                             start=(kf == 0), stop=(kf == KO - 1))
                    nc.vector.tensor_scalar_add(
                        out=dst[:, mo, :], in0=p, scalar1=bfm[:, mo:mo + 1])

            # V4: per head [v0..v63, 1] — the ones column accumulates the
            # softmax denominator in the AV matmul (no V bias: folded to bo')
            V4 = qkv.tile([128, LO, HEADS, HD + 1], bf16, tag="V4", name=f"V4_{b}")
            nc.vector.memset(V4[:, :, :, HD:HD + 1], 1.0)
            wv = w_bf["wv"]
            for lo in range(LO):
                p = psum_s()
                for (c0, cw) in _PROJ_CHUNKS:
                    for kf in range(KO):
                        nc.tensor.matmul(p[:, c0:c0 + cw],
                                         lhsT=hT[:, kf, lo * 128:(lo + 1) * 128],
                                         rhs=wv[:, kf, c0:c0 + cw],
                                         start=(kf == 0), stop=(kf == KO - 1))
                nc.vector.tensor_copy(
                    out=V4[:, lo, :, 0:HD],
                    in_=p[:, :HID].rearrange("p (h d) -> p h d", d=HD))
            return QT, KT, V4

        def emit_attention(b, QT, KT, V4, filler):
            """Per (head, q-half): 4x [S-pair -> wide exp -> AV-pair], then an
            immediate PSUM drain and a deferred in-place normalize on Pool.
            `filler` is a deque of (due_step_fraction, unit); units pop when
            the attention loop reaches their due position."""
            AT = fmB.tile([128, KO, L], bf16, tag="AT", name=f"AT_{b}")
            n_steps = HEADS * 2 * 4
            step = 0
            for h in range(HEADS):
                j, base = h // 2, 64 * (h % 2)
                for nh in range(2):
                    av = psum_av()
                    for kp in range(4):
                        sp = psum_s()
                        pt = ptp.tile([128, 1024], bf16, tag="PT",
                                      name=f"pt{b}_{h}_{nh}_{kp}")
                        for i in range(2):
                            ko = 2 * kp + i
                            nc.tensor.matmul(
                                sp[:, i * 512:(i + 1) * 512],
                                lhsT=KT[base:base + 64, j, ko * 128:(ko + 1) * 128],
                                rhs=QT[base:base + 64, j, nh * 512:(nh + 1) * 512],
                                tile_position=(base, 0))
                        nc.scalar.activation(out=pt, in_=sp, func=AF.Exp,
                                             scale=0.125, bias=negc_col)
                        for i in range(2):
                            ko = 2 * kp + i
                            nc.tensor.matmul(
                                av[0:HD + 1, :],
                                lhsT=V4[:, ko, h, :], rhs=pt[:, i * 512:(i + 1) * 512],
                                start=(ko == 0), stop=(ko == LO - 1))
                        step += 1
                        while filler and filler[0][0] * n_steps <= step:
                            filler.popleft()[1]()
                    # drain PSUM immediately (frees av for the pipeline)
                    at_sl = AT[base:base + 64, j, nh * 512:(nh + 1) * 512]
                    nc.vector.tensor_copy(out=at_sl, in_=av[0:HD, :])
                    if nh == 0:
                        den_row = small.tile([1, L], bf16, tag="denrow", bufs=1,
                                             name=f"ds{b}_{h}")
                    nc.vector.tensor_copy(out=den_row[:, nh * 512:(nh + 1) * 512],
                                          in_=av[HD:HD + 1, :])
                # per-head: reciprocal + broadcast ride the parallel DMA
                # queues so no compute engine's queue waits cross-engine
                dsl = io["den_dram"].ap()[b, h, :]
                rsl = io["rec_dram"].ap()[b, h, :]
                nc.sync.dma_start(out=dsl[None, :], in_=den_row)
                dpk = small.tile([64, L // 64], bf16, tag="dpk", name=f"dpk{b}_{h}")
                nc.sync.dma_start(out=dpk, in_=dsl.rearrange("(p f) -> p f", p=64))
                with nc.allow_low_precision(reason="softmax denom recip bf16"):
                    nc.vector.reciprocal(out=dpk, in_=dpk)
                nc.sync.dma_start(out=rsl.rearrange("(p f) -> p f", p=64), in_=dpk)
                rb = rbp.tile([128, L], bf16, tag="rb", name=f"rb{b}_{h}")
                nc.sync.dma_start(
                    out=rb, in_=rsl[None, :].partition_broadcast(128)[:, 0, :])
                nc.gpsimd.tensor_mul(out=AT[base:base + 64, j, :],
                                     in0=AT[base:base + 64, j, :],
                                     in1=rb[base:base + 64, :])
            while filler:
                filler.popleft()[1]()
            return AT

        def make_tail_units(b, pro_st, AT):
            """Post-attention work for batch b: out-proj + residual + LN2
            stats, x2hat, h2T, MLP1, MLP2 + final residual + store."""
            st = {}
            units = []
            g_bc = pro_st["g"]
            wo = w_bf["wo"]
            mv2 = small.tile([128, LO, 2], f32, tag="mv2", name=f"mv2_{b}")

            def dma_xrl(lo):
                t = xs.tile([128, HID], f32, tag="xsl", bufs=3, name=f"xrl_{b}_{lo}")
                nc.sync.dma_start(out=t, in_=x_view[b, :, lo, :])
                st.setdefault("xrl", {})[lo] = t

            def dma_x2(lo, pfx):
                t = xs.tile([128, HID], bf16, tag="x2st", name=f"{pfx}_{b}_{lo}")
                nc.sync.dma_start(out=t, in_=x2_view[b, :, lo, :])
                st.setdefault("x2t", {})[lo] = t

            def op_pf():
                dma_xrl(0)

            def oproj_u(lo):
                x_rl = st["xrl"].pop(lo)
                if lo + 1 < LO:
                    dma_xrl(lo + 1)
                x2_lo = xs.tile([128, HID], bf16, tag="x2st", name=f"x2o_{b}_{lo}")
                for (c0, cw) in _PROJ_CHUNKS:
                    p = psum_b(cw)
                    for kf in range(KO):
                        nc.tensor.matmul(p, lhsT=AT[:, kf, lo * 128:(lo + 1) * 128],
                                         rhs=wo[:, kf, c0:c0 + cw],
                                         start=(kf == 0), stop=False)
                    nc.tensor.matmul(p, lhsT=ones_bf, rhs=bo2_row[:, c0:c0 + cw],
                                     start=False, stop=True)
                    gm = gmp.tile([128, 512], bf16, tag="gm", name=f"gmo_{b}_{lo}_{c0}")
                    nc.vector.tensor_mul(out=gm[:, :cw], in0=p,
                                         in1=g_bc["gmsa"][:, c0:c0 + cw])
                    nc.gpsimd.tensor_add(out=x2_lo[:, c0:c0 + cw],
                                         in0=x_rl[:, c0:c0 + cw], in1=gm[:, :cw])
                nc.sync.dma_start(out=x2_view[b, :, lo, :], in_=x2_lo)
                ln_stats(x2_lo, mv2, lo)

            def x2pf_u():
                st["rstd2"] = small.tile([128, 8], f32, tag="rstd2",
                                         name=f"rstd2_{b}")
                st["x2hat"] = fmC.tile([128, LO, HID], bf16, tag="fmC",
                                       name=f"x2hat_{b}")
                rsqrt_newton(st["rstd2"][:, 0:], mv2[:, 0:4, 1], 4)
                dma_x2(0, "x2h")
                dma_x2(1, "x2h")

            def x2h_u(k):
                if k == 1:
                    rsqrt_newton(st["rstd2"][:, 4:], mv2[:, 4:8, 1], 4)
                for i in range(2):
                    lo = 2 * k + i
                    nc.vector.tensor_scalar(out=st["x2hat"][:, lo, :],
                                            in0=st["x2t"].pop(lo),
                                            scalar1=mv2[:, lo, 0:1],
                                            scalar2=st["rstd2"][:, lo:lo + 1],
                                            op0=OP.subtract, op1=OP.mult)
                if k < 3:
                    dma_x2(2 * k + 2, "x2h")
                    dma_x2(2 * k + 3, "x2h")

            def h2T_u(kf):
                if "h2T" not in st:
                    st["h2T"] = fmC.tile([128, KO, L], bf16, tag="fmC",
                                         name=f"h2T_{b}")
                x2hat, h2T = st["x2hat"], st["h2T"]
                for lo4 in range(0, LO, 4):
                    p = psum_b().bitcast(bf16)
                    for i in range(4):
                        nc.tensor.transpose(p[:, i * 128:(i + 1) * 128],
                                            x2hat[:, lo4 + i, kf * 128:(kf + 1) * 128],
                                            id_bf)
                    nc.vector.tensor_scalar(
                        out=h2T[:, kf, lo4 * 128:(lo4 + 4) * 128],
                        in0=p[:, :512],
                        scalar1=a2[:, kf, b:b + 1], scalar2=c2[:, kf, b:b + 1],
                        op0=OP.mult, op1=OP.add)

            def mlp1_u(mo, nh):
                if "m1T" not in st:
                    st["m1T"] = fmC.tile([128, KO, L], bf16, tag="fmC",
                                         name=f"m1T_{b}")
                m1T, h2T = st["m1T"], st["h2T"]
                w1 = w_bf["w1"]
                p = psum_b()
                for kf in range(KO):
                    nc.tensor.matmul(p, lhsT=w1[:, kf, mo * 128:(mo + 1) * 128],
                                     rhs=h2T[:, kf, nh * 512:(nh + 1) * 512],
                                     start=(kf == 0), stop=(kf == KO - 1))
                # silu(v) = 0.5*v*(tanh(v/2) + 1), v = p + b1
                th = gmp.tile([128, 512], bf16, tag="th", name=f"th_{b}_{mo}_{nh}")
                nc.scalar.activation(out=th, in_=p, func=AF.Tanh,
                                     scale=0.5, bias=b1h_fm[:, mo:mo + 1])
                vb = gmp.tile([128, 512], bf16, tag="vb", name=f"vb_{b}_{mo}_{nh}")
                nc.scalar.activation(out=vb, in_=p, func=AF.Identity,
                                     scale=0.5, bias=b1h_fm[:, mo:mo + 1])
                nc.gpsimd.tensor_add(out=th, in0=th,
                                     in1=ones_col_bf.to_broadcast([128, 512]))
                nc.gpsimd.tensor_mul(out=m1T[:, mo, nh * 512:(nh + 1) * 512],
                                     in0=vb, in1=th)

            def m2pf_u():
                dma_x2(0, "x2m")

            def mlp2_u(lo):
                m1T = st["m1T"]
                w2 = w_bf["w2"]
                x2_lo = st["x2t"].pop(lo)
                if lo + 1 < LO:
                    dma_x2(lo + 1, "x2m")
                o_st = outp.tile([128, HID], f32, tag="ost", name=f"ost_{b}_{lo}")
                for (c0, cw) in _PROJ_CHUNKS:
                    p = psum_b(cw)
                    for kf in range(KO):
                        nc.tensor.matmul(p, lhsT=m1T[:, kf, lo * 128:(lo + 1) * 128],
                                         rhs=w2[:, kf, c0:c0 + cw],
                                         start=(kf == 0), stop=False)
                    nc.tensor.matmul(p, lhsT=ones_bf, rhs=b2_row[:, c0:c0 + cw],
                                     start=False, stop=True)
                    gm = gmp.tile([128, 512], bf16, tag="gm",
                                  name=f"gmm_{b}_{lo}_{c0}")
                    nc.vector.tensor_mul(out=gm[:, :cw], in0=p,
                                         in1=g_bc["gmlp"][:, c0:c0 + cw])
                    nc.gpsimd.tensor_add(out=o_st[:, c0:c0 + cw],
                                         in0=x2_lo[:, c0:c0 + cw],
                                         in1=gm[:, :cw])
                nc.sync.dma_start(out=out_view[b, :, lo, :], in_=o_st)

            # due positions respect the dependency staircase:
            # oproj -> x2h -> h2T -> mlp1 -> mlp2
            units.append((0.035, op_pf))
            for lo in range(LO):
                units.append((0.05 + lo * 0.022, lambda lo=lo: oproj_u(lo)))
            units.append((0.24, x2pf_u))
            for k in range(4):
                units.append((0.27 + k * 0.028, lambda k=k: x2h_u(k)))
            for kf in range(KO):
                units.append((0.40 + kf * 0.022, lambda kf=kf: h2T_u(kf)))
            for i, (mo, nh) in enumerate((mo, nh) for mo in range(KO)
                                         for nh in range(2)):
                units.append((0.54 + i * 0.018,
                              lambda mo=mo, nh=nh: mlp1_u(mo, nh)))
            units.append((0.76, m2pf_u))
            for lo in range(LO):
                units.append((0.78 + lo * 0.024, lambda lo=lo: mlp2_u(lo)))
            return units

        # ================= main schedule =================
        pro_st, pro_u = make_pro_units(0)
        for _, u in pro_u:
            u()
        tail_q = []
        for b in range(nb):
            QT, KT, V4 = emit_qkv(b, pro_st)
            merged = list(tail_q)
            tail_q = []
            if b + 1 < nb:
                next_st, next_u = make_pro_units(b + 1)
                merged.extend(next_u)
            merged.sort(key=lambda du: du[0])
            # hoist the earliest units (x prefetches + first stats — DMA/DVE
            # only) to emit during the QKV block: their DMA latency is then
            # covered by the block's PE work instead of stalling window start
            merged = deque(merged)
            while merged and merged[0][0] <= 0.06:
                merged.popleft()[1]()
            AT = emit_attention(b, QT, KT, V4, merged)
            tail_q.extend(make_tail_units(b, pro_st, AT))
            if b + 1 < nb:
                pro_st = next_st
        for _, u in tail_q:
            u()


_nc_cache = {}


def _get_nc(nb=NB, L=L_FULL):
    key = (nb, L)
    if key not in _nc_cache:
        _nc_cache[key] = build_nc(nb, L)
    return _nc_cache[key]


def kernel(**inputs):
    from concourse.bass_utils import run_bass_kernel_spmd

    nc = _get_nc()
    per_core = []
    for c in range(NCORES):
        m = {}
        for name, arr in inputs.items():
            arr = np.asarray(arr, dtype=np.float32)
            if name in ("x_img", "cond"):
                m[name] = np.ascontiguousarray(arr[c * NB:(c + 1) * NB])
            else:
                m[name] = arr
        per_core.append(m)
    res = run_bass_kernel_spmd(nc, per_core, core_ids=list(range(NCORES)))
    return np.concatenate([res.results[c]["out"] for c in range(NCORES)], axis=0)


# revision 40
# speedup vs baseline: 1.0294x; 1.0294x over previous
"""DiT block kernel for Trainium2 (8 NeuronCores, data-parallel over batch).

Reference computation (per batch b):
    c = silu(cond) @ w_ada + b_ada
    shift_msa, scale_msa, gate_msa, shift_mlp, scale_mlp, gate_mlp = split(c)
    h  = LN1(x) * (1+scale_msa) + shift_msa
    x  = x + gate_msa * (attn(h) @ wo + bo)
    h2 = LN2(x) * (1+scale_mlp) + shift_mlp
    x  = x + gate_mlp * (silu(h2 @ w1 + b1) @ w2 + b2)

Per-core layout (8 batches, deeply software-pipelined):
  - matmul activations feature-major bf16 [128 = feat%128, 6, L]
  - attention per (head, q-half): S^T = K Q^T, two ko-tiles packed into one
    [128,1024] PSUM pair, one wide exp on ACT, AV with an appended ones
    column accumulating the softmax denominator -> per-(h,nh) reciprocal
    + DMA partition-broadcast + normalize multiply.
  - V bias folded into the out-projection bias (rows of attn sum to 1).
  - all non-attention work (LN prologue of b+1, out-proj/LN2/MLP of b-1)
    is emitted as ~45 "filler" units interleaved into attention(b) so the
    PE never sees a phase boundary; Q/K/V runs as a PE-dense block between
    attention phases.
"""

import numpy as np

HID = 768
HEADS = 12
HD = 64
B, L_FULL = 64, 1024
NCORES = 8
NB = B // NCORES
EPS = 1e-6

_PROJ_CHUNKS = ((0, 512), (512, 256))


def build_nc(nb=NB, L=L_FULL):
    import concourse.mybir as mybir
    import concourse.tile as tile
    from concourse import bacc

    f32 = mybir.dt.float32
    bf16 = mybir.dt.bfloat16

    nc = bacc.Bacc("TRN2", target_bir_lowering=False, debug=False)

    io = {}
    io["x_img"] = nc.dram_tensor("x_img", [nb, L, HID], f32, kind="ExternalInput")
    io["cond"] = nc.dram_tensor("cond", [nb, HID], f32, kind="ExternalInput")
    for name in ("wq", "wk", "wv", "wo", "w1", "w2"):
        io[name] = nc.dram_tensor(name, [HID, HID], f32, kind="ExternalInput")
    for name in ("bq", "bk", "bv", "bo", "b1", "b2"):
        io[name] = nc.dram_tensor(name, [HID], f32, kind="ExternalInput")
    io["w_ada"] = nc.dram_tensor("w_ada", [HID, 6 * HID], f32, kind="ExternalInput")
    io["b_ada"] = nc.dram_tensor("b_ada", [6 * HID], f32, kind="ExternalInput")
    for name in ("ln1_scale", "ln1_bias", "ln2_scale", "ln2_bias"):
        io[name] = nc.dram_tensor(name, [HID], f32, kind="ExternalInput")
    io["out"] = nc.dram_tensor("out", [nb, L, HID], f32, kind="ExternalOutput")
    io["c_dram"] = nc.dram_tensor("c_scratch", [nb, 6 * HID], f32)
    io["x2_dram"] = nc.dram_tensor("x2_scratch", [nb, L, HID], bf16)
    io["den_dram"] = nc.dram_tensor("den_scratch", [nb, HEADS, L], bf16)
    io["rec_dram"] = nc.dram_tensor("rec_scratch", [nb, HEADS, L], bf16)

    with tile.TileContext(nc) as tc:
        _build(tc, nc, io, nb, L)
    nc.compile()
    return nc


def _build(tc, nc, io, nb, L):
    import contextlib
    from collections import deque

    import concourse.mybir as mybir
    from concourse.masks import make_identity

    dt = mybir.dt
    f32, bf16 = dt.float32, dt.bfloat16
    AF = mybir.ActivationFunctionType
    OP = mybir.AluOpType

    LO = L // 128
    KO = HID // 128
    NADA = 6 * HID

    ctx = contextlib.ExitStack()
    with ctx:
        consts = ctx.enter_context(tc.tile_pool(name="consts", bufs=1))
        wpool = ctx.enter_context(tc.tile_pool(name="wpool", bufs=1))
        rows = ctx.enter_context(tc.tile_pool(name="rows", bufs=1))
        xs = ctx.enter_context(tc.tile_pool(name="xs", bufs=2))
        xhp = ctx.enter_context(tc.tile_pool(name="xhp", bufs=1))
        fmA = ctx.enter_context(tc.tile_pool(name="fmA", bufs=1))
        fmB = ctx.enter_context(tc.tile_pool(name="fmB", bufs=2))
        fmC = ctx.enter_context(tc.tile_pool(name="fmC", bufs=2))
        qkv = ctx.enter_context(tc.tile_pool(name="qkv", bufs=1))
        ptp = ctx.enter_context(tc.tile_pool(name="ptp", bufs=2))
        rbp = ctx.enter_context(tc.tile_pool(name="rbp", bufs=2))
        gmp = ctx.enter_context(tc.tile_pool(name="gmp", bufs=2))
        outp = ctx.enter_context(tc.tile_pool(name="outp", bufs=1))
        small = ctx.enter_context(tc.tile_pool(name="small", bufs=2))
        ps = ctx.enter_context(tc.tile_pool(name="ps", bufs=4, space="PSUM"))

        _psc = [0]

        def psum_b(w=512):
            _psc[0] += 1
            t = ps.tile([128, 512], f32, tag="ps_b", bufs=2, name=f"psb{_psc[0]}")
            return t[:, :w] if w != 512 else t

        def psum_s():
            _psc[0] += 1
            return ps.tile([128, 1024], f32, tag="ps_s", bufs=2, name=f"pss{_psc[0]}")

        def psum_av():
            _psc[0] += 1
            return ps.tile([128, 512], f32, tag="ps_av", bufs=2, name=f"psav{_psc[0]}")

        def stage2k():
            # 2KB/partition staging tiles time-share the attention pt slots
            _psc[0] += 1
            return ptp.tile([128, 512], f32, tag="PT", name=f"stg{_psc[0]}")

        # ---- constants ----
        id_bf = consts.tile([128, 128], bf16)
        make_identity(nc, id_bf)
        id_f32 = consts.tile([128, 128], f32)
        make_identity(nc, id_f32)
        ones_bf = consts.tile([1, 128], bf16)
        nc.vector.memset(ones_bf, 1.0)
        ones_f32 = consts.tile([1, 128], f32)
        nc.vector.memset(ones_f32, 1.0)
        ones_col_bf = consts.tile([128, 1], bf16)
        nc.vector.memset(ones_col_bf, 1.0)
        # constant subtracted inside exp (cancels in softmax)
        negc_col = consts.tile([128, 1], f32)
        nc.vector.memset(negc_col, -10.0)

        def load_fm(dram_vec):
            t = consts.tile([128, KO], f32, tag=f"fm_{dram_vec.name}")
            with nc.allow_non_contiguous_dma(reason="small 1d fm load"):
                nc.sync.dma_start(out=t, in_=dram_vec.ap().rearrange("(ko p) -> p ko", p=128))
            return t

        ln1s_fm = load_fm(io["ln1_scale"])
        ln1b_fm = load_fm(io["ln1_bias"])
        ln2s_fm = load_fm(io["ln2_scale"])
        ln2b_fm = load_fm(io["ln2_bias"])
        bq_fm = load_fm(io["bq"])
        bk_fm = load_fm(io["bk"])
        bv_fm = load_fm(io["bv"])
        b1_fm = load_fm(io["b1"])
        b1h_fm = consts.tile([128, KO], f32)  # 0.5 * b1, bias for tanh(x/2)
        nc.vector.tensor_scalar_mul(out=b1h_fm, in0=b1_fm, scalar1=0.5)
        bv_bfc = consts.tile([128, KO], bf16)  # bf16 feature-major bv column
        nc.vector.tensor_copy(out=bv_bfc, in_=bv_fm)

        bo_r32 = rows.tile([1, HID], f32, tag="row_f32", name="bo_r32")
        nc.sync.dma_start(out=bo_r32, in_=io["bo"].ap()[None, :])
        bo_rowb = consts.tile([1, HID], bf16, tag="bo_rowb")
        nc.vector.tensor_copy(out=bo_rowb, in_=bo_r32)
        b2_r32 = rows.tile([1, HID], f32, tag="row_f32", name="b2_r32")
        nc.sync.dma_start(out=b2_r32, in_=io["b2"].ap()[None, :])
        b2_row = consts.tile([1, HID], bf16, tag="b2_row")
        nc.vector.tensor_copy(out=b2_row, in_=b2_r32)

        # ---- conditioning: scT = silu(cond)^T [128, KO, nb] via a
        # transposed (feature-major) DMA load of the small cond tensor ----
        condT = consts.tile([128, KO, nb], f32)
        condT_view = io["cond"].ap().rearrange("b (ko p) -> p ko b", p=128)
        with nc.allow_non_contiguous_dma(reason="small cond transposed load"):
            for kf in range(KO):
                nc.sync.dma_start(out=condT[:, kf, :], in_=condT_view[:, kf, :])
        sgT = consts.tile([128, KO, nb], f32)
        nc.scalar.activation(out=sgT, in_=condT, func=AF.Sigmoid)
        scT = consts.tile([128, KO, nb], f32)
        nc.vector.tensor_mul(out=scT, in0=condT, in1=sgT)

        # ---- c = silu(cond) @ w_ada + b_ada  -> c_dram + feature-major cT ----
        cT = consts.tile([128, 6 * KO, nb], f32)
        wada_view = io["w_ada"].ap().rearrange("(ko p) n -> p ko n", p=128)
        for jc in range(NADA // 512):
            bst = rows.tile([1, 512], f32, tag="row_f32")
            nc.sync.dma_start(out=bst, in_=io["b_ada"].ap()[None, jc * 512:(jc + 1) * 512])
            pc = psum_b()
            for kf in range(KO):
                wst = stage2k()
                nc.sync.dma_start(out=wst,
                                  in_=wada_view[:, kf, jc * 512:(jc + 1) * 512])
                nc.tensor.matmul(pc[:nb, :], lhsT=scT[:, kf, :], rhs=wst,
                                 start=(kf == 0), stop=False)
            nc.tensor.matmul(pc[:nb, :], lhsT=ones_f32[:, :nb],
                             rhs=bst, start=False, stop=True)
            cst = stage2k()[:nb, :]
            nc.vector.tensor_copy(out=cst, in_=pc[:nb, :])
            nc.sync.dma_start(out=io["c_dram"].ap()[:, jc * 512:(jc + 1) * 512], in_=cst)
            for mt in range(4):
                mo = jc * 4 + mt
                ptr = psum_b()
                nc.tensor.transpose(ptr[:, :nb], cst[:, mt * 128:(mt + 1) * 128],
                                    id_f32[:nb, :nb])
                nc.vector.tensor_copy(out=cT[:, mo, :], in_=ptr[:, :nb])

        def chunk(i):
            return cT[:, 6 * i:6 * i + 6, :]

        a1 = consts.tile([128, KO, nb], f32)
        c1 = consts.tile([128, KO, nb], f32)
        a2 = consts.tile([128, KO, nb], f32)
        c2 = consts.tile([128, KO, nb], f32)
        tmp_m = consts.tile([128, KO, nb], f32)
        for (a, c, lns, lnb, sc_i, sh_i) in ((a1, c1, ln1s_fm, ln1b_fm, 1, 0),
                                             (a2, c2, ln2s_fm, ln2b_fm, 4, 3)):
            nc.vector.tensor_scalar_add(out=tmp_m, in0=chunk(sc_i), scalar1=1.0)
            nc.vector.tensor_mul(out=a, in0=tmp_m,
                                 in1=lns[:, :, None].to_broadcast([128, KO, nb]))
            nc.vector.tensor_mul(out=c, in0=tmp_m,
                                 in1=lnb[:, :, None].to_broadcast([128, KO, nb]))
            nc.vector.tensor_add(out=c, in0=c, in1=chunk(sh_i))

        # ---- weights -> SBUF bf16 [128, KO, 768]; q/k/v now, the rest
        # (wo/w1/w2, first needed by batch-0 tail during attention(1)) are
        # streamed during attention(0) to shorten the startup serial phase ----
        w_bf = {}

        def load_weight(name):
            wt = wpool.tile([128, KO, HID], bf16, tag=f"w_{name}")
            w_view = io[name].ap().rearrange("(ko p) n -> p ko n", p=128)
            for kf in range(KO):
                for (c0, cw) in _PROJ_CHUNKS:
                    st = stage2k()
                    nc.sync.dma_start(out=st[:, :cw], in_=w_view[:, kf, c0:c0 + cw])
                    nc.vector.tensor_copy(out=wt[:, kf, c0:c0 + cw], in_=st[:, :cw])
            w_bf[name] = wt

        for name in ("wq", "wk", "wv"):
            load_weight(name)

        # ---- bo' = bv @ wo + bo (attention rows sum to 1, so the V bias
        # passes through AV untouched and folds into the out-proj bias);
        # emitted after attention(0), once wo has streamed in ----
        bo2_row = consts.tile([1, HID], bf16, tag="bo2_row")

        def emit_bo2():
            for (c0, cw) in _PROJ_CHUNKS:
                p = psum_b(cw)
                for kf in range(KO):
                    nc.tensor.matmul(p[:1, :], lhsT=bv_bfc[:, kf:kf + 1],
                                     rhs=w_bf["wo"][:, kf, c0:c0 + cw],
                                     start=(kf == 0), stop=False)
                nc.tensor.matmul(p[:1, :], lhsT=ones_bf[:, :1],
                                 rhs=bo_rowb[:, c0:c0 + cw], start=False, stop=True)
                nc.vector.tensor_copy(out=bo2_row[:, c0:c0 + cw], in_=p[:1, :])

        # ---- helpers ----
        def rsqrt_newton(dst, var_ap, n):
            """dst[:, :n] = 1/sqrt(var + EPS); Newton from seed 1.0 (var~1)."""
            vt = small.tile([128, 8], f32, tag="rs_v")
            nc.vector.tensor_scalar_add(out=vt[:, :n], in0=var_ap, scalar1=EPS)
            hv = small.tile([128, 8], f32, tag="rs_h")
            nc.vector.tensor_scalar_mul(out=hv[:, :n], in0=vt[:, :n], scalar1=0.5)
            nc.vector.memset(dst[:, :n], 1.0)
            tt = small.tile([128, 8], f32, tag="rs_t")
            for _ in range(5):
                nc.vector.tensor_mul(out=tt[:, :n], in0=dst[:, :n], in1=dst[:, :n])
                nc.vector.tensor_mul(out=tt[:, :n], in0=tt[:, :n], in1=hv[:, :n])
                nc.vector.tensor_scalar(out=tt[:, :n], in0=tt[:, :n],
                                        scalar1=-1.0, scalar2=1.5, op0=OP.mult, op1=OP.add)
                nc.vector.tensor_mul(out=dst[:, :n], in0=dst[:, :n], in1=tt[:, :n])

        def ln_stats(src, mv, lo):
            stats = small.tile([128, 3, 6], f32, tag="stats")
            for s in range(3):
                nc.vector.bn_stats(out=stats[:, s, :], in_=src[:, s * 256:(s + 1) * 256])
            nc.vector.bn_aggr(out=mv[:, lo, :], in_=stats)

        x_view = io["x_img"].ap().rearrange("b (lo p) d -> b p lo d", p=128)
        out_view = io["out"].ap().rearrange("b (lo p) d -> b p lo d", p=128)
        x2_view = io["x2_dram"].ap().rearrange("b (lo p) d -> b p lo d", p=128)

        # ================= per-batch unit generators =================

        def make_pro_units(b):
            """LN1 prologue for batch b. Every DMA is issued one unit ahead
            of the compute that reads it, so DVE ops never wait at the queue
            head (which would block later-queued PSUM drains and stall PE)."""
            st = {"xt": {}}

            def gates_u():
                g_bc = {}
                for gi, nm in ((2, "gmsa"), (5, "gmlp")):
                    gr = rows.tile([1, HID], f32, tag="row_f32", name=f"gr_{b}_{nm}")
                    nc.sync.dma_start(out=gr, in_=io["c_dram"].ap()[b:b + 1, gi * HID:(gi + 1) * HID])
                    grb = rows.tile([1, HID], bf16, tag="growb", name=f"grb_{b}_{nm}")
                    nc.vector.tensor_copy(out=grb, in_=gr)
                    gb = small.tile([128, HID], bf16, tag=f"gbc_{nm}", bufs=2,
                                    name=f"gb_{b}_{nm}")
                    nc.gpsimd.partition_broadcast(gb, grb, channels=128)
                    g_bc[nm] = gb
                st["g"] = g_bc

            def dma_x(lo, pfx):
                t = xs.tile([128, HID], f32, tag="xsl", bufs=3, name=f"{pfx}_{b}_{lo}")
                nc.sync.dma_start(out=t, in_=x_view[b, :, lo, :])
                st["xt"][lo] = t

            def pfx_u():
                st["mv1"] = small.tile([128, LO, 2], f32, tag="mv1", name=f"mv1_{b}")
                st["rstd1"] = small.tile([128, 8], f32, tag="rstd1", name=f"rstd1_{b}")
                dma_x(0, "xs")
                dma_x(1, "xs")

            def st_u(k):
                for lo in (2 * k, 2 * k + 1):
                    ln_stats(st["xt"].pop(lo), st["mv1"], lo)
                if k < 3:
                    dma_x(2 * k + 2, "xs")
                    dma_x(2 * k + 3, "xs")
                if k == 1:
                    rsqrt_newton(st["rstd1"][:, 0:], st["mv1"][:, 0:4, 1], 4)
                if k == 3:
                    rsqrt_newton(st["rstd1"][:, 4:], st["mv1"][:, 4:8, 1], 4)

            def xh_u(k):
                # k=0: prefetch only; k>=1: xhat(2k-2, 2k-1) + prefetch next
                if k >= 1:
                    g = (k - 1) // 2
                    if (k - 1) % 2 == 0:
                        st["xhg"] = xhp.tile([128, 4, HID], bf16, tag="xhg",
                                             name=f"xhg_{b}_{g}")
                    xg = st["xhg"]
                    for i in range(2):
                        lo = 2 * (k - 1) + i
                        nc.vector.tensor_scalar(out=xg[:, lo - 4 * g, :],
                                                in0=st["xt"].pop(lo),
                                                scalar1=st["mv1"][:, lo, 0:1],
                                                scalar2=st["rstd1"][:, lo:lo + 1],
                                                op0=OP.subtract, op1=OP.mult)
                if k < 4:
                    dma_x(2 * k, "xh")
                    dma_x(2 * k + 1, "xh")

            def hT_u(g, half):
                if "hT" not in st:
                    st["hT"] = fmA.tile([128, KO, L], bf16, tag="fmA", name=f"hT_{b}")
                xg, hT = st["xhg"], st["hT"]
                for kf in range(3 * half, 3 * half + 3):
                    p = psum_b().bitcast(bf16)
                    for i in range(4):
                        nc.tensor.transpose(p[:, i * 128:(i + 1) * 128],
                                            xg[:, i, kf * 128:(kf + 1) * 128], id_bf)
                    nc.vector.tensor_scalar(
                        out=hT[:, kf, g * 512:(g + 1) * 512], in0=p[:, :512],
                        scalar1=a1[:, kf, b:b + 1], scalar2=c1[:, kf, b:b + 1],
                        op0=OP.mult, op1=OP.add)

            units = [(0.97, gates_u), (0.02, pfx_u),
                     (0.05, lambda: st_u(0)), (0.08, lambda: st_u(1)),
                     (0.11, lambda: st_u(2)), (0.14, lambda: st_u(3)),
                     (0.16, lambda: xh_u(0)), (0.18, lambda: xh_u(1)),
                     (0.21, lambda: xh_u(2)),
                     (0.24, lambda: hT_u(0, 0)), (0.27, lambda: hT_u(0, 1)),
                     (0.30, lambda: xh_u(3)), (0.33, lambda: xh_u(4)),
                     (0.36, lambda: hT_u(1, 0)), (0.39, lambda: hT_u(1, 1))]
            return st, units

        def emit_qkv(b, pro_st):
            """Q/K/V projections for batch b (PE-dense block)."""
            hT = pro_st["hT"]
            QT = qkv.tile([128, KO, L], bf16, tag="QT", name=f"QT_{b}")
            KT = qkv.tile([128, KO, L], bf16, tag="KT", name=f"KT_{b}")
            for (dst, wname, bfm) in ((QT, "wq", bq_fm), (KT, "wk", bk_fm)):
                wt = w_bf[wname]
                for mo in range(KO):
                    p = psum_s()
                    for nh in range(2):
                        for kf in range(KO):
                            nc.tensor.matmul(p[:, nh * 512:(nh + 1) * 512],
                                             lhsT=wt[:, kf, mo * 128:(mo + 1) * 128],
                                             rhs=hT[:, kf, nh * 512:(nh + 1) * 512],
                                             start=(kf == 0), stop=(kf == KO - 1))
                    nc.vector.tensor_scalar_add(
                        out=dst[:, mo, :], in0=p, scalar1=bfm[:, mo:mo + 1])

            # V4: per head [v0..v63, 1] — the ones column accumulates the
            # softmax denominator in the AV matmul (no V bias: folded to bo')
            V4 = qkv.tile([128, LO, HEADS, HD + 1], bf16, tag="V4", name=f"V4_{b}")
            nc.vector.memset(V4[:, :, :, HD:HD + 1], 1.0)
            wv = w_bf["wv"]
            for lo in range(LO):
                p = psum_s()
                for (c0, cw) in _PROJ_CHUNKS:
                    for kf in range(KO):
                        nc.tensor.matmul(p[:, c0:c0 + cw],
                                         lhsT=hT[:, kf, lo * 128:(lo + 1) * 128],
                                         rhs=wv[:, kf, c0:c0 + cw],
                                         start=(kf == 0), stop=(kf == KO - 1))
                nc.vector.tensor_copy(
                    out=V4[:, lo, :, 0:HD],
                    in_=p[:, :HID].rearrange("p (h d) -> p h d", d=HD))
            return QT, KT, V4

        def emit_attention(b, QT, KT, V4, filler):
            """Per (head, q-half): 4x [S-pair -> wide exp -> AV-pair], then an
            immediate PSUM drain and a deferred in-place normalize on Pool.
            `filler` is a deque of (due_step_fraction, unit); units pop when
            the attention loop reaches their due position."""
            AT = fmB.tile([128, KO, L], bf16, tag="AT", name=f"AT_{b}")
            n_steps = HEADS * 2 * 4
            step = 0
            for h in range(HEADS):
                j, base = h // 2, 64 * (h % 2)
                for nh in range(2):
                    av = psum_av()
                    for kp in range(4):
                        sp = psum_s()
                        pt = ptp.tile([128, 1024], bf16, tag="PT",
                                      name=f"pt{b}_{h}_{nh}_{kp}")
                        for i in range(2):
                            ko = 2 * kp + i
                            nc.tensor.matmul(
                                sp[:, i * 512:(i + 1) * 512],
                                lhsT=KT[base:base + 64, j, ko * 128:(ko + 1) * 128],
                                rhs=QT[base:base + 64, j, nh * 512:(nh + 1) * 512],
                                tile_position=(base, 0))
                        nc.scalar.activation(out=pt, in_=sp, func=AF.Exp,
                                             scale=0.125, bias=negc_col)
                        for i in range(2):
                            ko = 2 * kp + i
                            nc.tensor.matmul(
                                av[0:HD + 1, :],
                                lhsT=V4[:, ko, h, :], rhs=pt[:, i * 512:(i + 1) * 512],
                                start=(ko == 0), stop=(ko == LO - 1))
                        step += 1
                        while filler and filler[0][0] * n_steps <= step:
                            filler.popleft()[1]()
                    # drain PSUM immediately (frees av for the pipeline)
                    at_sl = AT[base:base + 64, j, nh * 512:(nh + 1) * 512]
                    nc.vector.tensor_copy(out=at_sl, in_=av[0:HD, :])
                    if nh == 0:
                        den_row = small.tile([1, L], bf16, tag="denrow", bufs=1,
                                             name=f"ds{b}_{h}")
                    nc.vector.tensor_copy(out=den_row[:, nh * 512:(nh + 1) * 512],
                                          in_=av[HD:HD + 1, :])
                # per-head: reciprocal + broadcast ride the parallel DMA
                # queues so no compute engine's queue waits cross-engine
                dsl = io["den_dram"].ap()[b, h, :]
                rsl = io["rec_dram"].ap()[b, h, :]
                nc.sync.dma_start(out=dsl[None, :], in_=den_row)
                dpk = small.tile([64, L // 64], bf16, tag="dpk", name=f"dpk{b}_{h}")
                nc.sync.dma_start(out=dpk, in_=dsl.rearrange("(p f) -> p f", p=64))
                with nc.allow_low_precision(reason="softmax denom recip bf16"):
                    nc.vector.reciprocal(out=dpk, in_=dpk)
                nc.sync.dma_start(out=rsl.rearrange("(p f) -> p f", p=64), in_=dpk)
                rb = rbp.tile([128, L], bf16, tag="rb", name=f"rb{b}_{h}")
                nc.sync.dma_start(
                    out=rb, in_=rsl[None, :].partition_broadcast(128)[:, 0, :])
                nc.gpsimd.tensor_mul(out=AT[base:base + 64, j, :],
                                     in0=AT[base:base + 64, j, :],
                                     in1=rb[base:base + 64, :])
            while filler:
                filler.popleft()[1]()
            return AT

        def make_tail_units(b, pro_st, AT):
            """Post-attention work for batch b: out-proj + residual + LN2
            stats, x2hat, h2T, MLP1, MLP2 + final residual + store."""
            st = {}
            units = []
            g_bc = pro_st["g"]
            wo = w_bf["wo"]
            mv2 = small.tile([128, LO, 2], f32, tag="mv2", name=f"mv2_{b}")

            def dma_xrl(lo):
                t = xs.tile([128, HID], f32, tag="xsl", bufs=3, name=f"xrl_{b}_{lo}")
                nc.sync.dma_start(out=t, in_=x_view[b, :, lo, :])
                st.setdefault("xrl", {})[lo] = t

            def dma_x2(lo, pfx):
                t = xs.tile([128, HID], bf16, tag="x2st", name=f"{pfx}_{b}_{lo}")
                nc.sync.dma_start(out=t, in_=x2_view[b, :, lo, :])
                st.setdefault("x2t", {})[lo] = t

            def op_pf():
                dma_xrl(0)

            def oproj_u(lo):
                x_rl = st["xrl"].pop(lo)
                if lo + 1 < LO:
                    dma_xrl(lo + 1)
                x2_lo = xs.tile([128, HID], bf16, tag="x2st", name=f"x2o_{b}_{lo}")
                for (c0, cw) in _PROJ_CHUNKS:
                    p = psum_b(cw)
                    for kf in range(KO):
                        nc.tensor.matmul(p, lhsT=AT[:, kf, lo * 128:(lo + 1) * 128],
                                         rhs=wo[:, kf, c0:c0 + cw],
                                         start=(kf == 0), stop=False)
                    nc.tensor.matmul(p, lhsT=ones_bf, rhs=bo2_row[:, c0:c0 + cw],
                                     start=False, stop=True)
                    gm = gmp.tile([128, 512], bf16, tag="gm", name=f"gmo_{b}_{lo}_{c0}")
                    nc.vector.tensor_mul(out=gm[:, :cw], in0=p,
                                         in1=g_bc["gmsa"][:, c0:c0 + cw])
                    nc.gpsimd.tensor_add(out=x2_lo[:, c0:c0 + cw],
                                         in0=x_rl[:, c0:c0 + cw], in1=gm[:, :cw])
                nc.sync.dma_start(out=x2_view[b, :, lo, :], in_=x2_lo)
                ln_stats(x2_lo, mv2, lo)

            def x2pf_u():
                st["rstd2"] = small.tile([128, 8], f32, tag="rstd2",
                                         name=f"rstd2_{b}")
                st["x2hat"] = fmC.tile([128, LO, HID], bf16, tag="fmC",
                                       name=f"x2hat_{b}")
                rsqrt_newton(st["rstd2"][:, 0:], mv2[:, 0:4, 1], 4)
                dma_x2(0, "x2h")
                dma_x2(1, "x2h")

            def x2h_u(k):
                if k == 1:
                    rsqrt_newton(st["rstd2"][:, 4:], mv2[:, 4:8, 1], 4)
                for i in range(2):
                    lo = 2 * k + i
                    nc.vector.tensor_scalar(out=st["x2hat"][:, lo, :],
                                            in0=st["x2t"].pop(lo),
                                            scalar1=mv2[:, lo, 0:1],
                                            scalar2=st["rstd2"][:, lo:lo + 1],
                                            op0=OP.subtract, op1=OP.mult)
                if k < 3:
                    dma_x2(2 * k + 2, "x2h")
                    dma_x2(2 * k + 3, "x2h")

            def h2T_u(kf):
                if "h2T" not in st:
                    st["h2T"] = fmC.tile([128, KO, L], bf16, tag="fmC",
                                         name=f"h2T_{b}")
                x2hat, h2T = st["x2hat"], st["h2T"]
                for lo4 in range(0, LO, 4):
                    p = psum_b().bitcast(bf16)
                    for i in range(4):
                        nc.tensor.transpose(p[:, i * 128:(i + 1) * 128],
                                            x2hat[:, lo4 + i, kf * 128:(kf + 1) * 128],
                                            id_bf)
                    nc.vector.tensor_scalar(
                        out=h2T[:, kf, lo4 * 128:(lo4 + 4) * 128],
                        in0=p[:, :512],
                        scalar1=a2[:, kf, b:b + 1], scalar2=c2[:, kf, b:b + 1],
                        op0=OP.mult, op1=OP.add)

            def mlp1_u(mo, nh):
                if "m1T" not in st:
                    st["m1T"] = fmC.tile([128, KO, L], bf16, tag="fmC",
                                         name=f"m1T_{b}")
                m1T, h2T = st["m1T"], st["h2T"]
                w1 = w_bf["w1"]
                p = psum_b()
                for kf in range(KO):
                    nc.tensor.matmul(p, lhsT=w1[:, kf, mo * 128:(mo + 1) * 128],
                                     rhs=h2T[:, kf, nh * 512:(nh + 1) * 512],
                                     start=(kf == 0), stop=(kf == KO - 1))
                # silu(v) = 0.5*v*(tanh(v/2) + 1), v = p + b1
                th = gmp.tile([128, 512], bf16, tag="th", name=f"th_{b}_{mo}_{nh}")
                nc.scalar.activation(out=th, in_=p, func=AF.Tanh,
                                     scale=0.5, bias=b1h_fm[:, mo:mo + 1])
                vb = gmp.tile([128, 512], bf16, tag="vb", name=f"vb_{b}_{mo}_{nh}")
                nc.scalar.activation(out=vb, in_=p, func=AF.Identity,
                                     scale=0.5, bias=b1h_fm[:, mo:mo + 1])
                nc.gpsimd.tensor_add(out=th, in0=th,
                                     in1=ones_col_bf.to_broadcast([128, 512]))
                nc.gpsimd.tensor_mul(out=m1T[:, mo, nh * 512:(nh + 1) * 512],
                                     in0=vb, in1=th)

            def m2pf_u():
                dma_x2(0, "x2m")

            def mlp2_u(lo):
                m1T = st["m1T"]
                w2 = w_bf["w2"]
                x2_lo = st["x2t"].pop(lo)
                if lo + 1 < LO:
                    dma_x2(lo + 1, "x2m")
                o_st = outp.tile([128, HID], f32, tag="ost", name=f"ost_{b}_{lo}")
                for (c0, cw) in _PROJ_CHUNKS:
                    p = psum_b(cw)
                    for kf in range(KO):
                        nc.tensor.matmul(p, lhsT=m1T[:, kf, lo * 128:(lo + 1) * 128],
                                         rhs=w2[:, kf, c0:c0 + cw],
                                         start=(kf == 0), stop=False)
                    nc.tensor.matmul(p, lhsT=ones_bf, rhs=b2_row[:, c0:c0 + cw],
                                     start=False, stop=True)
                    gm = gmp.tile([128, 512], bf16, tag="gm",
                                  name=f"gmm_{b}_{lo}_{c0}")
                    nc.vector.tensor_mul(out=gm[:, :cw], in0=p,
                                         in1=g_bc["gmlp"][:, c0:c0 + cw])
                    nc.gpsimd.tensor_add(out=o_st[:, c0:c0 + cw],
                                         in0=x2_lo[:, c0:c0 + cw],
                                         in1=gm[:, :cw])
                nc.sync.dma_start(out=out_view[b, :, lo, :], in_=o_st)

            # due positions respect the dependency staircase:
            # oproj -> x2h -> h2T -> mlp1 -> mlp2
            units.append((0.035, op_pf))
            for lo in range(LO):
                units.append((0.05 + lo * 0.022, lambda lo=lo: oproj_u(lo)))
            units.append((0.24, x2pf_u))
            for k in range(4):
                units.append((0.27 + k * 0.028, lambda k=k: x2h_u(k)))
            for kf in range(KO):
                units.append((0.40 + kf * 0.022, lambda kf=kf: h2T_u(kf)))
            for i, (mo, nh) in enumerate((mo, nh) for mo in range(KO)
                                         for nh in range(2)):
                units.append((0.54 + i * 0.018,
                              lambda mo=mo, nh=nh: mlp1_u(mo, nh)))
            units.append((0.76, m2pf_u))
            for lo in range(LO):
                units.append((0.78 + lo * 0.024, lambda lo=lo: mlp2_u(lo)))
            return units

        # ================= main schedule =================
        pro_st, pro_u = make_pro_units(0)
        for _, u in pro_u:
            u()
        tail_q = []
        for b in range(nb):
            QT, KT, V4 = emit_qkv(b, pro_st)
            if b == 0:
                for name in ("wo", "w1", "w2"):
                    load_weight(name)
            merged = list(tail_q)
            tail_q = []
            if b + 1 < nb:
                next_st, next_u = make_pro_units(b + 1)
                merged.extend(next_u)
            merged.sort(key=lambda du: du[0])
            AT = emit_attention(b, QT, KT, V4, deque(merged))
            if b == 0:
                emit_bo2()
            tail_q.extend(make_tail_units(b, pro_st, AT))
            if b + 1 < nb:
                pro_st = next_st
        for _, u in tail_q:
            u()


_nc_cache = {}


def _get_nc(nb=NB, L=L_FULL):
    key = (nb, L)
    if key not in _nc_cache:
        _nc_cache[key] = build_nc(nb, L)
    return _nc_cache[key]


def kernel(**inputs):
    from concourse.bass_utils import run_bass_kernel_spmd

    nc = _get_nc()
    per_core = []
    for c in range(NCORES):
        m = {}
        for name, arr in inputs.items():
            arr = np.asarray(arr, dtype=np.float32)
            if name in ("x_img", "cond"):
                m[name] = np.ascontiguousarray(arr[c * NB:(c + 1) * NB])
            else:
                m[name] = arr
        per_core.append(m)
    res = run_bass_kernel_spmd(nc, per_core, core_ids=list(range(NCORES)))
    return np.concatenate([res.results[c]["out"] for c in range(NCORES)], axis=0)
